# revision 19
# baseline (speedup 1.0000x reference)
"""Trainium2 Bass kernel for nn_AGTLayer (GAT-style additive-attention layer).

Algebraic collapse: softmax_j(sl[i] + sr[j]) is independent of i, so the
attention reduces to one weighted mean per (batch, head):
    p[j]  = exp(sr[j]) / sum_j exp(sr[j])
    c[h]  = sum_j p[j] fr[j, h*128:(h+1)*128]
    fh    = concat_h(c[h]) @ Wf.T          (ONE vector per batch)
    out   = LayerNorm(h + fh[None, :])

Sharding: core c handles batch b = c//2 REDUNDANTLY (both cores of a pair
compute the full-batch attention; no collectives at all), then applies the
LayerNorm epilogue to its own half of the rows (half = c%2).

Speed tricks:
 - fp8(e4m3) DoubleRow matmuls everywhere on the big GEMM: K=256 per pass,
   ~2x bf16 column rate. h, Wr, Wf shipped pre-transposed/blocked in fp8.
 - att_r is folded into Wr's columns on the host (sigma_d = G*sign(a_d)*
   max(|a_d|, EPS)), so fr comes out of the matmul pre-scaled (t2 = sigma*fr);
   1/sigma is folded into Wf rows. The linear score term sum_d a_d*fr_d is an
   8-column matmul against host-computed WA; the leaky-relu |.| term is ONE
   vector stt (abs_max then * sign-const) + per-head reduce:
       sr = 0.505*sum(a*fr) + 0.495*sum(sign(a)*|t2|)/G
 - softmax denominator accumulated on-chip (sacc += w), collapsed to s with a
   single tiny matmul against ones; w = exp(sr - 1.5) (shift-invariant).
 - LayerNorm epilogue split across gpsimd/vector (adds+stats) and
   scalar/gpsimd (normalize apply); outputs stream on 3 DMA queues.
"""

import numpy as np
import ml_dtypes
from contextlib import ExitStack

import concourse.bass as bass
import concourse.mybir as mybir
import concourse.tile as tile
from concourse import bacc
from concourse.bass_utils import run_bass_kernel_spmd

AF = mybir.ActivationFunctionType
ALU = mybir.AluOpType
DR = mybir.MatmulPerfMode.DoubleRow
F32 = mybir.dt.float32
BF16 = mybir.dt.bfloat16
FP8 = mybir.dt.float8e4

B, N, D, H, HD = 4, 2048, 1024, 8, 128
NCORES = 8
P = 128
MT = N // P            # 16 row-tiles of the full batch per core
MTO = 8                # 8 output row-tiles (this core's half)
KP = D // 256          # 4 k-pairs (256 contraction per DoubleRow pass)
NB = 512               # psum bank free-dim (f32)
LN_EPS = 1e-5
G = 16.0               # column scale folded into Wr (and 1/G into Wf)
EPS_A = 0.012          # |att| clamp so Wf2 rows stay in fp8 range
W_BIAS = -1.5          # softmax shift (invariant)
NP8 = ml_dtypes.float8_e4m3
NBF = ml_dtypes.bfloat16


def _bcast_ap(ap, parts, free):
    return bass.AP(tensor=ap.tensor, offset=ap.offset, ap=[[0, parts], [1, free]])


def _build(apply_gb: bool):
    nc = bacc.Bacc(
        "TRN2",
        target_bir_lowering=False,
        debug=False,
        enable_asserts=False,
        num_devices=NCORES,
    )

    hTm = nc.dram_tensor("hTm", [N, D], FP8, kind="ExternalInput")
    wr = nc.dram_tensor("wr", [P, 8 * D], FP8, kind="ExternalInput")
    wa = nc.dram_tensor("wa", [P, 8 * 16], FP8, kind="ExternalInput")
    wf = nc.dram_tensor("wf", [P, 8 * D], FP8, kind="ExternalInput")
    hF = nc.dram_tensor("hF", [MTO * P, D], BF16, kind="ExternalInput")
    sgn = nc.dram_tensor("sgn", [1, D], BF16, kind="ExternalInput")
    hs = nc.dram_tensor("hs", [P, MTO], F32, kind="ExternalInput")
    out = nc.dram_tensor("out", [MTO * P, D], BF16, kind="ExternalOutput")
    if apply_gb:
        gam = nc.dram_tensor("gam", [1, D], F32, kind="ExternalInput")
        bet = nc.dram_tensor("bet", [1, D], F32, kind="ExternalInput")

    with tile.TileContext(nc) as tc, ExitStack() as ctx:
        const = ctx.enter_context(tc.tile_pool(name="const", bufs=1))
        work = ctx.enter_context(tc.tile_pool(name="work", bufs=3))
        ep = ctx.enter_context(tc.tile_pool(name="ep", bufs=4))
        eps_p = ctx.enter_context(tc.tile_pool(name="eps", bufs=4))
        frp = ctx.enter_context(tc.tile_pool(name="frp", bufs=2, space="PSUM"))
        stp = ctx.enter_context(tc.tile_pool(name="stp", bufs=2, space="PSUM"))
        up = ctx.enter_context(tc.tile_pool(name="up", bufs=1, space="PSUM"))
        dram = ctx.enter_context(tc.tile_pool(name="dram", bufs=1, space="DRAM"))

        # ---- tiny constants ----
        warm_w = const.tile([P, NB], BF16, tag="warm_w")
        nc.vector.memset(warm_w[:], 0.0)
        ones_m = const.tile([P, 1], BF16, tag="ones_m")
        nc.vector.memset(ones_m[:], 1.0)
        ones1b = const.tile([1, P], BF16, tag="ones1b")
        nc.vector.memset(ones1b[:], 1.0)
        eps_sb = const.tile([P, 1], F32, tag="eps")
        nc.vector.memset(eps_sb[:], LN_EPS)
        sacc = const.tile([P, H], F32, tag="sacc")
        nc.gpsimd.memset(sacc[:], 0.0)
        wb_sb = const.tile([P, 1], F32, tag="wb")
        nc.vector.memset(wb_sb[:], W_BIAS)
        zero_sb = const.tile([P, 1], F32, tag="zero")
        nc.vector.memset(zero_sb[:], 0.0)

        # ---- PE warmup burst: unthrottle HAM while first DMAs land ----
        warm_ps = frp.tile([P, D], F32, tag="fr")
        NWARM = 9
        for i in range(NWARM):
            nc.tensor.matmul(warm_ps[:, 0:NB], lhsT=warm_w[:, 0:P], rhs=warm_w[:],
                             start=(i == 0), stop=(i == NWARM - 1))

        # ---- input loads, spread across DGE queues ----
        # critical path: wr_kp0 halves + htm0 + wa + sgn feed tile 0.
        wr_t, wf_t, htm, h_t = [], [], [], []
        for kp in range(KP):
            wr_t.append(const.tile([P, 2, D], FP8, tag=f"wr{kp}", name=f"wr{kp}"))
            wf_t.append(const.tile([P, 2, D], FP8, tag=f"wf{kp}", name=f"wf{kp}"))
        for mt in range(MT):
            htm.append(const.tile([P, H * P], FP8, tag=f"htm{mt}", name=f"htm{mt}"))
        for mt in range(MTO):
            h_t.append(const.tile([P, D], BF16, tag=f"h{mt}", name=f"h{mt}"))
        wa_sb = const.tile([P, 8 * 16], FP8, tag="wa")
        hs_sb = const.tile([P, MTO], F32, tag="hs")
        sgn_b = const.tile([P, D], BF16, tag="sgn")

        wr_src = wr.ap().rearrange("p (kp ks n) -> p kp ks n", kp=KP, ks=2)
        wf_src = wf.ap().rearrange("p (kp ks n) -> p kp ks n", kp=KP, ks=2)
        # tile 0 needs ALL wr chunks + htm0 + wa + sgn: front-load those on
        # all 3 queues, then stream htm in consumption order, then wf/hF.
        engs = [nc.sync, nc.scalar, nc.gpsimd]
        nc.gpsimd.dma_start(out=wa_sb[:], in_=wa.ap())
        nc.gpsimd.dma_start(out=sgn_b[:], in_=_bcast_ap(sgn.ap(), P, D))
        nc.sync.dma_start(out=wr_t[0][:], in_=wr_src[:, 0])
        nc.scalar.dma_start(out=htm[0][:], in_=hTm.ap()[0:P, :])
        nc.gpsimd.dma_start(out=wr_t[1][:], in_=wr_src[:, 1])
        nc.sync.dma_start(out=wr_t[2][:], in_=wr_src[:, 2])
        nc.scalar.dma_start(out=wr_t[3][:], in_=wr_src[:, 3])
        for mt in range(1, MT):
            engs[mt % 3].dma_start(out=htm[mt][:], in_=hTm.ap()[mt * P:(mt + 1) * P, :])
        nc.sync.dma_start(out=hs_sb[:], in_=hs.ap())
        for kp in range(KP):
            engs[kp % 3].dma_start(out=wf_t[kp][:], in_=wf_src[:, kp])
        for mt in range(MTO):
            engs[(mt + 1) % 3].dma_start(out=h_t[mt][:], in_=hF.ap()[mt * P:(mt + 1) * P, :])
        if apply_gb:
            gam_sb = const.tile([P, D], F32, tag="gam")
            nc.sync.dma_start(out=gam_sb[:], in_=_bcast_ap(gam.ap(), P, D))
            bet_sb = const.tile([P, D], F32, tag="bet")
            nc.sync.dma_start(out=bet_sb[:], in_=_bcast_ap(bet.ap(), P, D))

        # ---- main loop: fr + scores, u accumulated one tile behind ----
        frb2 = [const.tile([P, 2, D], FP8, tag=f"frb{p}", name=f"frb{p}") for p in range(MT // 2)]
        w2 = [const.tile([P, 2, 16], FP8, tag=f"w{p}", name=f"w{p}") for p in range(MT // 2)]
        u_ps = up.tile([H, D], F32, tag="u")

        def u_mms(pr):
            lw = w2[pr][:, :, 0:H]
            for nh in range(2):
                nc.tensor.matmul(
                    u_ps[0:H, nh * NB:(nh + 1) * NB],
                    lhsT=lw,
                    rhs=frb2[pr][:, :, nh * NB:(nh + 1) * NB],
                    start=(pr == 0), stop=(pr == MT // 2 - 1),
                    perf_mode=DR,
                )

        for step in range(MT + 1):
            if step < MT:
                mt = step
                pr, ko = mt // 2, mt % 2
                fr = frp.tile([P, D], F32, tag="fr")
                sT = stp.tile([P, H], F32, tag="sT")
                htm_r = htm[mt][:].rearrange("p (ks j) -> p ks j", ks=H)
                wa_r = wa_sb[:].rearrange("p (s c) -> p s c", c=16)
                for kp in range(KP):
                    lhs = htm_r[:, 2 * kp:2 * kp + 2, :]
                    for nh in range(2):
                        nc.tensor.matmul(
                            fr[:, nh * NB:(nh + 1) * NB],
                            lhsT=lhs,
                            rhs=wr_t[kp][:, :, nh * NB:(nh + 1) * NB],
                            start=(kp == 0), stop=(kp == KP - 1),
                            perf_mode=DR,
                        )
                    nc.tensor.matmul(
                        sT[:, 0:H],
                        lhsT=lhs,
                        rhs=wa_r[:, 2 * kp:2 * kp + 2, 0:H],
                        start=(kp == 0), stop=(kp == KP - 1),
                        perf_mode=DR,
                    )
                # evac fr -> fp8 (scalar), |t2|*sgn (gpsimd/vector), reduce,
                # combine with linear term, exp -> w (scalar), sacc += w.
                nc.scalar.activation(out=frb2[pr][:, ko, :], in_=fr[:], func=AF.Copy)
                m = work.tile([P, D], BF16, tag="m")
                if mt % 2 == 0:
                    nc.scalar.activation(out=m[:], in_=fr[:], func=AF.Abs,
                                         bias=zero_sb[:, 0:1])
                else:
                    fb = frb2[pr][:, ko, :]
                    nc.vector.scalar_tensor_tensor(out=m[:], in0=fb,
                                                   scalar=-1.0, in1=fb,
                                                   op0=ALU.mult, op1=ALU.max)
                ms = work.tile([P, D], BF16, tag="ms")
                nc.gpsimd.tensor_tensor(out=ms[:], in0=m[:], in1=sgn_b[:],
                                        op=ALU.mult)
                S = work.tile([P, H], F32, tag="S")
                nc.vector.tensor_reduce(
                    out=S[:],
                    in_=ms[:].rearrange("p (h hd) -> p h hd", h=H),
                    axis=mybir.AxisListType.X,
                    op=ALU.add,
                )
                sr = work.tile([P, H], F32, tag="sr")
                nc.vector.scalar_tensor_tensor(out=sr[:], in0=sT[:],
                                               scalar=1.0 / G, in1=S[:],
                                               op0=ALU.mult, op1=ALU.add)
                nc.scalar.activation(out=w2[pr][:, ko, 0:H], in_=sr[:],
                                     func=AF.Exp, bias=wb_sb[:, 0:1])
                nc.vector.tensor_tensor(out=sacc[:], in0=sacc[:],
                                        in1=w2[pr][:, ko, 0:H], op=ALU.add)
            if step >= 2 and step % 2 == 0:
                u_mms((step - 2) // 2)
        u_mms(MT // 2 - 1)

        # ---- softmax denominator + normalized context ----
        sacc_bf = const.tile([P, H], BF16, tag="sacc_bf")
        nc.vector.tensor_copy(out=sacc_bf[:], in_=sacc[:])
        s_ps = stp.tile([H, 1], F32, tag="sT")
        nc.tensor.matmul(s_ps[:], lhsT=sacc_bf[:], rhs=ones_m[:],
                         start=True, stop=True)
        rs = const.tile([H, 1], F32, tag="rs")
        nc.vector.reciprocal(out=rs[:], in_=s_ps[:])
        un_sb = const.tile([H, D], FP8, tag="un_sb")
        nc.vector.tensor_scalar(out=un_sb[:], in0=u_ps[:], scalar1=rs[:, 0:1],
                                scalar2=None, op0=ALU.mult)
        cdr = dram.tile([1, D], FP8, tag="cdr")
        for hh in range(H):
            engs[hh % 3].dma_start(out=cdr[0:1, hh * HD:(hh + 1) * HD],
                                   in_=un_sb[hh:hh + 1, hh * HD:(hh + 1) * HD])
        cn_t = [const.tile([P, 2 * 16], FP8, tag=f"cn{kp}", name=f"cn{kp}")
                for kp in range(KP)]
        for kp in range(KP):
            cb = cdr[0:1, kp * 2 * HD:kp * 2 * HD + 1]
            dst = cn_t[kp][:].rearrange("p (k s) -> p k s", s=16)
            engs[kp % 3].dma_start(
                out=dst[:, :, 0],
                in_=bass.AP(tensor=cb.tensor, offset=cb.offset,
                            ap=[[1, P], [P, 2]]),
            )

        # ---- fh = c @ Wf2, broadcast to 128 rows ----
        fh_ps = frp.tile([1, D], F32, tag="fr")
        for kp in range(KP):
            lhs = cn_t[kp][:].rearrange("p (k s) -> p k s", s=16)[:, :, 0:1]
            for nh in range(2):
                nc.tensor.matmul(
                    fh_ps[0:1, nh * NB:(nh + 1) * NB],
                    lhsT=lhs,
                    rhs=wf_t[kp][:, :, nh * NB:(nh + 1) * NB],
                    start=(kp == 0), stop=(kp == KP - 1),
                    perf_mode=DR,
                )
        fh_sb = const.tile([1, D], BF16, tag="fh_sb")
        nc.vector.tensor_copy(out=fh_sb[:], in_=fh_ps[:])
        fhb_ps = frp.tile([P, D], F32, tag="fr")
        for nh in range(2):
            nc.tensor.matmul(fhb_ps[:, nh * NB:(nh + 1) * NB], lhsT=ones1b[:],
                             rhs=fh_sb[0:1, nh * NB:(nh + 1) * NB],
                             start=True, stop=True)
        fhb = const.tile([P, D], BF16, tag="fhb")
        nc.scalar.activation(out=fhb[:, 0:NB], in_=fhb_ps[:, 0:NB], func=AF.Copy)
        nc.vector.tensor_copy(out=fhb[:, NB:D], in_=fhb_ps[:, NB:D])

        # ---- epilogue (bn_stats variant for bisect) ----
        mv_all = eps_p.tile([P, MTO, 2], F32, tag="mv_all")
        sd_all = eps_p.tile([P, MTO], F32, tag="sd_all")
        rstd_all = eps_p.tile([P, MTO], F32, tag="rstd_all")
        nmr_all = eps_p.tile([P, MTO], F32, tag="nmr_all")
        y_t = [None] * MTO
        HALF = MTO // 2
        dma_engs = [nc.sync, nc.gpsimd, nc.scalar]
        for half in range(2):
            lo = half * HALF
            for mt in range(lo, lo + HALF):
                y = ep.tile([P, D], BF16, tag=f"y{mt % 4}")
                aeng = nc.vector if mt % 2 == 0 else nc.gpsimd
                aeng.tensor_tensor(out=y[:], in0=h_t[mt][:], in1=fhb[:],
                                   op=ALU.add)
                st = eps_p.tile([P, 2, 6], F32, tag="st")
                nc.vector.bn_stats(out=st[:, 0, :], in_=y[:, 0:NB])
                nc.vector.bn_stats(out=st[:, 1, :], in_=y[:, NB:D])
                nc.vector.bn_aggr(out=mv_all[:, mt, :], in_=st[:])
                y_t[mt] = y
            hs_sl = slice(lo, lo + HALF)
            nc.scalar.activation(out=sd_all[:, hs_sl], in_=mv_all[:, hs_sl, 1],
                                 func=AF.Sqrt, bias=eps_sb[:])
            nc.vector.reciprocal(out=rstd_all[:, hs_sl], in_=sd_all[:, hs_sl])
            nc.vector.scalar_tensor_tensor(out=nmr_all[:, hs_sl],
                                           in0=mv_all[:, hs_sl, 0],
                                           scalar=-1.0, in1=rstd_all[:, hs_sl],
                                           op0=ALU.mult, op1=ALU.mult)
            for mt in range(lo, lo + HALF):
                o = ep.tile([P, D], BF16, tag="o")
                if mt % 2 == 0:
                    nc.scalar.activation(out=o[:], in_=y_t[mt][:],
                                         func=AF.Identity,
                                         scale=rstd_all[:, mt:mt + 1],
                                         bias=nmr_all[:, mt:mt + 1])
                else:
                    nc.gpsimd.tensor_scalar(out=o[:], in0=y_t[mt][:],
                                            scalar1=rstd_all[:, mt:mt + 1],
                                            scalar2=nmr_all[:, mt:mt + 1],
                                            op0=ALU.mult, op1=ALU.add)
                if apply_gb:
                    nc.vector.tensor_tensor(out=o[:], in0=o[:], in1=gam_sb[:],
                                            op=ALU.mult)
                    nc.vector.tensor_tensor(out=o[:], in0=o[:], in1=bet_sb[:],
                                            op=ALU.add)
                dma_engs[mt % 3].dma_start(out=out.ap()[mt * P:(mt + 1) * P, :],
                                           in_=o[:])

    nc.compile()
    return nc


_NC_CACHE = {}


def _get_nc(apply_gb: bool):
    if apply_gb not in _NC_CACHE:
        _NC_CACHE[apply_gb] = _build(apply_gb)
    return _NC_CACHE[apply_gb]


def _prep_weights(Wr, att_r, Wf):
    a = np.asarray(att_r, np.float32).reshape(HD)
    at = np.tile(a, H)                            # a_d, d = 0..1023
    sg = np.where(at >= 0, 1.0, -1.0).astype(np.float32)
    sig = sg * np.maximum(np.abs(at), EPS_A) * G  # sigma_d

    WrT = np.ascontiguousarray(np.asarray(Wr, np.float32).T)   # [k, d]
    Wrp = WrT * sig[None, :]
    wr_host = np.ascontiguousarray(
        Wrp.reshape(KP, 2, P, D).transpose(2, 0, 1, 3).reshape(P, 8 * D)
    ).astype(NP8)

    # wa layout: sbuf tile [P, 8(kp*2+ks), 16] with cols 0..7 = heads
    wa_k = np.zeros((D, 16), np.float32)
    for hh in range(H):
        wa_k[:, hh] = 0.505 * G * (WrT[:, hh * HD:(hh + 1) * HD] @ a)
    wa_host = np.ascontiguousarray(
        wa_k.reshape(KP, 2, P, 16).transpose(2, 0, 1, 3).reshape(P, 8 * 16)
    ).astype(NP8)

    WfT = np.ascontiguousarray(np.asarray(Wf, np.float32).T)   # [d, n]
    Wf2 = WfT / sig[:, None]
    wf_host = np.ascontiguousarray(
        Wf2.reshape(KP, 2, P, D).transpose(2, 0, 1, 3).reshape(P, 8 * D)
    ).astype(NP8)

    sgn_host = ((0.495 / G) * sg).reshape(1, D).astype(NBF)
    return wr_host, wa_host, wf_host, sgn_host


def _make_in_maps(h, Wr, att_r, Wf, ln_gamma, ln_beta, apply_gb):
    wr_host, wa_host, wf_host, sgn_host = _prep_weights(Wr, att_r, Wf)
    hf = np.asarray(h, np.float32)                # [B, N, D]
    in_maps = []
    for c in range(NCORES):
        b, half = c // 2, c % 2
        hb = hf[b]                                # [2048, 1024]
        hT = hb.T.reshape(KP, 2, P, MT, P).transpose(3, 2, 0, 1, 4)
        hTm = np.ascontiguousarray(hT.reshape(N, D)).astype(NP8)
        m = {
            "hTm": hTm,
            "wr": wr_host,
            "wa": wa_host,
            "wf": wf_host,
            "hF": np.ascontiguousarray(
                hb[half * MTO * P:(half + 1) * MTO * P]).astype(NBF),
            "hs": np.ascontiguousarray(
                hb[half * MTO * P:(half + 1) * MTO * P].sum(axis=1)
                .reshape(MTO, P).T),
            "sgn": sgn_host,
        }
        if apply_gb:
            m["gam"] = np.asarray(ln_gamma, np.float32).reshape(1, D)
            m["bet"] = np.asarray(ln_beta, np.float32).reshape(1, D)
        in_maps.append(m)
    return in_maps


def _run(h, Wl, Wr, att_l, att_r, Wf, ln_gamma, ln_beta, trace=False):
    g = np.asarray(ln_gamma, np.float32)
    bta = np.asarray(ln_beta, np.float32)
    apply_gb = not (np.all(g == 1.0) and np.all(bta == 0.0))
    nc = _get_nc(apply_gb)
    in_maps = _make_in_maps(h, Wr, att_r, Wf, ln_gamma, ln_beta, apply_gb)
    res = run_bass_kernel_spmd(nc, in_maps, core_ids=list(range(NCORES)),
                               trace=trace)
    outs = [np.asarray(res.results[c]["out"], np.float32) for c in range(NCORES)]
    full = np.concatenate(outs, axis=0).reshape(B, N, D)
    return full, res


def kernel(**inputs):
    out, _ = _run(**inputs)
    return out


# revision 26
# speedup vs baseline: 1.2177x; 1.2177x over previous
"""Trainium2 Bass kernel for nn_AGTLayer (GAT-style additive-attention layer).

Algebraic collapse: softmax_j(sl[i] + sr[j]) is independent of i, so the
attention reduces to one weighted mean per (batch, head):
    p[j]  = exp(sr[j]) / sum_j exp(sr[j])
    c[h]  = sum_j p[j] fr[j, h*128:(h+1)*128]
    fh    = concat_h(c[h]) @ Wf.T          (ONE vector per batch)
    out   = LayerNorm(h + fh[None, :])

Sharding: core c handles batch b = c//2 REDUNDANTLY (both cores of a pair
compute the full-batch attention; no collectives at all), then applies the
LayerNorm epilogue to its own half of the rows (half = c%2).

Speed tricks:
 - fp8(e4m3) DoubleRow matmuls everywhere on the big GEMM: K=256 per pass,
   ~2x bf16 column rate. h, Wr, Wf shipped pre-transposed/blocked in fp8.
 - att_r is folded into Wr's columns on the host (sigma_d = G*sign(a_d)*
   max(|a_d|, EPS)), so fr comes out of the matmul pre-scaled (t2 = sigma*fr);
   1/sigma is folded into Wf rows. The linear score term sum_d a_d*fr_d is an
   8-column matmul against host-computed WA; the leaky-relu |.| term is ONE
   vector stt (abs_max then * sign-const) + per-head reduce:
       sr = 0.505*sum(a*fr) + 0.495*sum(sign(a)*|t2|)/G
 - softmax denominator accumulated on-chip (sacc += w), collapsed to s with a
   single tiny matmul against ones; w = exp(sr - 1.5) (shift-invariant).
 - LayerNorm epilogue split across gpsimd/vector (adds+stats) and
   scalar/gpsimd (normalize apply); outputs stream on 3 DMA queues.
"""

import numpy as np
import ml_dtypes
from contextlib import ExitStack

import concourse.bass as bass
import concourse.mybir as mybir
import concourse.tile as tile
from concourse import bacc
from concourse.bass_utils import run_bass_kernel_spmd

AF = mybir.ActivationFunctionType
ALU = mybir.AluOpType
DR = mybir.MatmulPerfMode.DoubleRow
F32 = mybir.dt.float32
BF16 = mybir.dt.bfloat16
FP8 = mybir.dt.float8e4

B, N, D, H, HD = 4, 2048, 1024, 8, 128
NCORES = 8
P = 128
MT = N // P            # 16 row-tiles of the full batch per core
MTO = 8                # 8 output row-tiles (this core's half)
KP = D // 256          # 4 k-pairs (256 contraction per DoubleRow pass)
NB = 512               # psum bank free-dim (f32)
LN_EPS = 1e-5
G = 16.0               # column scale folded into Wr (and 1/G into Wf)
EPS_A = 0.012          # |att| clamp so Wf2 rows stay in fp8 range
W_BIAS = -1.5          # softmax shift (invariant)
NP8 = ml_dtypes.float8_e4m3
NBF = ml_dtypes.bfloat16


def _bcast_ap(ap, parts, free):
    return bass.AP(tensor=ap.tensor, offset=ap.offset, ap=[[0, parts], [1, free]])


def _build(apply_gb: bool):
    nc = bacc.Bacc(
        "TRN2",
        target_bir_lowering=False,
        debug=False,
        enable_asserts=False,
        num_devices=NCORES,
    )

    hTm = nc.dram_tensor("hTm", [N, D], FP8, kind="ExternalInput")
    wr = nc.dram_tensor("wr", [P, 8 * D], FP8, kind="ExternalInput")
    wa = nc.dram_tensor("wa", [P, 8 * 16], FP8, kind="ExternalInput")
    wf = nc.dram_tensor("wf", [P, 8 * D], FP8, kind="ExternalInput")
    hF = nc.dram_tensor("hF", [MTO * P, D], BF16, kind="ExternalInput")
    sgn = nc.dram_tensor("sgn", [1, D], BF16, kind="ExternalInput")
    hs = nc.dram_tensor("hs", [P, MTO], F32, kind="ExternalInput")
    id8 = nc.dram_tensor("id8", [P, P], BF16, kind="ExternalInput")
    out = nc.dram_tensor("out", [MTO * P, D], BF16, kind="ExternalOutput")
    if apply_gb:
        gam = nc.dram_tensor("gam", [1, D], F32, kind="ExternalInput")
        bet = nc.dram_tensor("bet", [1, D], F32, kind="ExternalInput")

    with tile.TileContext(nc) as tc, ExitStack() as ctx:
        const = ctx.enter_context(tc.tile_pool(name="const", bufs=1))
        work = ctx.enter_context(tc.tile_pool(name="work", bufs=3))
        ep = ctx.enter_context(tc.tile_pool(name="ep", bufs=4))
        eps_p = ctx.enter_context(tc.tile_pool(name="eps", bufs=4))
        frp = ctx.enter_context(tc.tile_pool(name="frp", bufs=2, space="PSUM"))
        stp = ctx.enter_context(tc.tile_pool(name="stp", bufs=2, space="PSUM"))
        up = ctx.enter_context(tc.tile_pool(name="up", bufs=1, space="PSUM"))
        dram = ctx.enter_context(tc.tile_pool(name="dram", bufs=1, space="DRAM"))

        # ---- tiny constants ----
        warm_w = const.tile([P, NB], BF16, tag="warm_w")
        nc.vector.memset(warm_w[:], 0.0)
        ones_m = const.tile([P, 1], BF16, tag="ones_m")
        nc.vector.memset(ones_m[:], 1.0)
        ones1b = const.tile([1, P], BF16, tag="ones1b")
        nc.vector.memset(ones1b[:], 1.0)
        eps_sb = const.tile([P, 1], F32, tag="eps")
        nc.vector.memset(eps_sb[:], LN_EPS)
        sacc = const.tile([P, H], F32, tag="sacc")
        nc.gpsimd.memset(sacc[:], 0.0)
        wb_sb = const.tile([P, 1], F32, tag="wb")
        nc.vector.memset(wb_sb[:], W_BIAS)
        zero_sb = const.tile([P, 1], F32, tag="zero")
        nc.vector.memset(zero_sb[:], 0.0)
        un_sb = const.tile([P, D], BF16, tag="un_sb")
        nc.gpsimd.memset(un_sb[:], 0.0)

        # ---- PE warmup burst: unthrottle HAM while first DMAs land ----
        warm_ps = frp.tile([P, D], F32, tag="fr")
        NWARM = 9
        for i in range(NWARM):
            nc.tensor.matmul(warm_ps[:, 0:NB], lhsT=warm_w[:, 0:P], rhs=warm_w[:],
                             start=(i == 0), stop=(i == NWARM - 1))

        # ---- input loads, spread across DGE queues ----
        # critical path: wr_kp0 halves + htm0 + wa + sgn feed tile 0.
        wr_t, wf_t, htm, h_t = [], [], [], []
        for kp in range(KP):
            wr_t.append(const.tile([P, 2, D], FP8, tag=f"wr{kp}", name=f"wr{kp}"))
            wf_t.append(const.tile([P, 2, D], FP8, tag=f"wf{kp}", name=f"wf{kp}"))
        for mt in range(MT):
            htm.append(const.tile([P, H * P], FP8, tag=f"htm{mt}", name=f"htm{mt}"))
        for mt in range(MTO):
            h_t.append(const.tile([P, D], BF16, tag=f"h{mt}", name=f"h{mt}"))
        wa_sb = const.tile([P, 8 * 16], FP8, tag="wa")
        hs_sb = const.tile([P, MTO], F32, tag="hs")
        id_sb = const.tile([P, P], BF16, tag="id8")
        sgn_b = const.tile([P, D], BF16, tag="sgn")

        wr_src = wr.ap().rearrange("p (kp ks n) -> p kp ks n", kp=KP, ks=2)
        wf_src = wf.ap().rearrange("p (kp ks n) -> p kp ks n", kp=KP, ks=2)
        # tile 0 needs ALL wr chunks + htm0 + wa + sgn: front-load those on
        # all 3 queues, then stream htm in consumption order, then wf/hF.
        engs = [nc.sync, nc.scalar, nc.gpsimd]
        nc.gpsimd.dma_start(out=wa_sb[:], in_=wa.ap())
        nc.gpsimd.dma_start(out=sgn_b[:], in_=_bcast_ap(sgn.ap(), P, D))
        nc.sync.dma_start(out=wr_t[0][:], in_=wr_src[:, 0])
        nc.scalar.dma_start(out=htm[0][:], in_=hTm.ap()[0:P, :])
        nc.gpsimd.dma_start(out=wr_t[1][:], in_=wr_src[:, 1])
        nc.sync.dma_start(out=wr_t[2][:], in_=wr_src[:, 2])
        nc.scalar.dma_start(out=wr_t[3][:], in_=wr_src[:, 3])
        for mt in range(1, MT):
            engs[mt % 3].dma_start(out=htm[mt][:], in_=hTm.ap()[mt * P:(mt + 1) * P, :])
        nc.sync.dma_start(out=hs_sb[:], in_=hs.ap())
        nc.scalar.dma_start(out=id_sb[:], in_=id8.ap())
        for kp in range(KP):
            engs[kp % 3].dma_start(out=wf_t[kp][:], in_=wf_src[:, kp])
        for mt in range(MTO):
            engs[(mt + 1) % 3].dma_start(out=h_t[mt][:], in_=hF.ap()[mt * P:(mt + 1) * P, :])
        if apply_gb:
            gam_sb = const.tile([P, D], F32, tag="gam")
            nc.sync.dma_start(out=gam_sb[:], in_=_bcast_ap(gam.ap(), P, D))
            bet_sb = const.tile([P, D], F32, tag="bet")
            nc.sync.dma_start(out=bet_sb[:], in_=_bcast_ap(bet.ap(), P, D))

        # ---- main loop: fr + scores, u accumulated one tile behind ----
        frb2 = [const.tile([P, 2, D], FP8, tag=f"frb{p}", name=f"frb{p}") for p in range(MT // 2)]
        w2 = [const.tile([P, 2, 16], FP8, tag=f"w{p}", name=f"w{p}") for p in range(MT // 2)]
        u_ps = up.tile([H, D], F32, tag="u")

        def u_mms(pr):
            lw = w2[pr][:, :, 0:H]
            for nh in range(2):
                nc.tensor.matmul(
                    u_ps[0:H, nh * NB:(nh + 1) * NB],
                    lhsT=lw,
                    rhs=frb2[pr][:, :, nh * NB:(nh + 1) * NB],
                    start=(pr == 0), stop=(pr == MT // 2 - 1),
                    perf_mode=DR,
                )

        for step in range(MT + 1):
            if step < MT:
                mt = step
                pr, ko = mt // 2, mt % 2
                fr = frp.tile([P, D], F32, tag="fr")
                sT = stp.tile([P, H], F32, tag="sT")
                htm_r = htm[mt][:].rearrange("p (ks j) -> p ks j", ks=H)
                wa_r = wa_sb[:].rearrange("p (s c) -> p s c", c=16)
                for kp in range(KP):
                    lhs = htm_r[:, 2 * kp:2 * kp + 2, :]
                    for nh in range(2):
                        nc.tensor.matmul(
                            fr[:, nh * NB:(nh + 1) * NB],
                            lhsT=lhs,
                            rhs=wr_t[kp][:, :, nh * NB:(nh + 1) * NB],
                            start=(kp == 0), stop=(kp == KP - 1),
                            perf_mode=DR,
                        )
                    nc.tensor.matmul(
                        sT[:, 0:H],
                        lhsT=lhs,
                        rhs=wa_r[:, 2 * kp:2 * kp + 2, 0:H],
                        start=(kp == 0), stop=(kp == KP - 1),
                        perf_mode=DR,
                    )
                # evac fr -> fp8 (scalar), |t2|*sgn (gpsimd/vector), reduce,
                # combine with linear term, exp -> w (scalar), sacc += w.
                nc.scalar.activation(out=frb2[pr][:, ko, :], in_=fr[:], func=AF.Copy)
                m = work.tile([P, D], BF16, tag="m")
                if mt % 2 == 0:
                    nc.scalar.activation(out=m[:], in_=fr[:], func=AF.Abs,
                                         bias=zero_sb[:, 0:1])
                else:
                    fb = frb2[pr][:, ko, :]
                    nc.vector.scalar_tensor_tensor(out=m[:], in0=fb,
                                                   scalar=-1.0, in1=fb,
                                                   op0=ALU.mult, op1=ALU.max)
                ms = work.tile([P, D], BF16, tag="ms")
                nc.gpsimd.tensor_tensor(out=ms[:], in0=m[:], in1=sgn_b[:],
                                        op=ALU.mult)
                S = work.tile([P, H], F32, tag="S")
                nc.vector.tensor_reduce(
                    out=S[:],
                    in_=ms[:].rearrange("p (h hd) -> p h hd", h=H),
                    axis=mybir.AxisListType.X,
                    op=ALU.add,
                )
                sr = work.tile([P, H], F32, tag="sr")
                nc.vector.scalar_tensor_tensor(out=sr[:], in0=sT[:],
                                               scalar=1.0 / G, in1=S[:],
                                               op0=ALU.mult, op1=ALU.add)
                nc.scalar.activation(out=w2[pr][:, ko, 0:H], in_=sr[:],
                                     func=AF.Exp, bias=wb_sb[:, 0:1])
                nc.vector.tensor_tensor(out=sacc[:], in0=sacc[:],
                                        in1=w2[pr][:, ko, 0:H], op=ALU.add)
            if step >= 2 and step % 2 == 0:
                u_mms((step - 2) // 2)
        u_mms(MT // 2 - 1)

        # ---- softmax denominator + normalized context ----
        sacc_bf = const.tile([P, H], BF16, tag="sacc_bf")
        nc.vector.tensor_copy(out=sacc_bf[:], in_=sacc[:])
        s_ps = stp.tile([H, 1], F32, tag="sT")
        nc.tensor.matmul(s_ps[:], lhsT=sacc_bf[:], rhs=ones_m[:],
                         start=True, stop=True)
        rs = const.tile([H, 1], F32, tag="rs")
        nc.vector.reciprocal(out=rs[:], in_=s_ps[:])
        nc.vector.tensor_scalar(out=un_sb[0:H, :], in0=u_ps[:],
                                scalar1=rs[:, 0:1], scalar2=None, op0=ALU.mult)
        # transpose each 128-block of un via PE; head h's own block is column
        # h of transpose #h. Assemble fp8 lhsT tiles for the fh matmuls.
        cn_t = [const.tile([P, 2 * 16], FP8, tag=f"cn{kp}", name=f"cn{kp}")
                for kp in range(KP)]
        for hh in range(H):
            tp = stp.tile([P, P], BF16, tag="sT")
            nc.tensor.transpose(out=tp[:], in_=un_sb[:, hh * HD:(hh + 1) * HD],
                                identity=id_sb[:])
            dst = cn_t[hh // 2][:].rearrange("p (k s) -> p k s", s=16)
            nc.vector.tensor_copy(out=dst[:, hh % 2, 0:1], in_=tp[:, hh:hh + 1])

        # ---- fh = c @ Wf2, broadcast to 128 rows ----
        fh_ps = frp.tile([1, D], F32, tag="fr")
        for kp in range(KP):
            lhs = cn_t[kp][:].rearrange("p (k s) -> p k s", s=16)[:, :, 0:1]
            for nh in range(2):
                nc.tensor.matmul(
                    fh_ps[0:1, nh * NB:(nh + 1) * NB],
                    lhsT=lhs,
                    rhs=wf_t[kp][:, :, nh * NB:(nh + 1) * NB],
                    start=(kp == 0), stop=(kp == KP - 1),
                    perf_mode=DR,
                )
        fh_sb = const.tile([1, D], BF16, tag="fh_sb")
        nc.vector.tensor_copy(out=fh_sb[:], in_=fh_ps[:])
        fhb_ps = frp.tile([P, D], F32, tag="fr")
        for nh in range(2):
            nc.tensor.matmul(fhb_ps[:, nh * NB:(nh + 1) * NB], lhsT=ones1b[:],
                             rhs=fh_sb[0:1, nh * NB:(nh + 1) * NB],
                             start=True, stop=True)
        fhb = const.tile([P, D], BF16, tag="fhb")
        nc.scalar.activation(out=fhb[:, 0:NB], in_=fhb_ps[:, 0:NB], func=AF.Copy)
        nc.vector.tensor_copy(out=fhb[:, NB:D], in_=fhb_ps[:, NB:D])

        # ---- epilogue: per-tile pipeline (add -> stats -> rstd -> apply) ----
        mv_all = eps_p.tile([P, MTO, 2], F32, tag="mv_all")
        sd_all = eps_p.tile([P, MTO], F32, tag="sd_all")
        rstd_all = eps_p.tile([P, MTO], F32, tag="rstd_all")
        nmr_all = eps_p.tile([P, MTO], F32, tag="nmr_all")
        dma_engs = [nc.sync, nc.gpsimd, nc.scalar]
        for mt in range(MTO):
            y = ep.tile([P, D], BF16, tag=f"y{mt % 4}")
            aeng = nc.vector if mt % 2 == 0 else nc.gpsimd
            aeng.tensor_tensor(out=y[:], in0=h_t[mt][:], in1=fhb[:],
                               op=ALU.add)
            st = eps_p.tile([P, 2, 6], F32, tag="st")
            nc.vector.bn_stats(out=st[:, 0, :], in_=y[:, 0:NB])
            nc.vector.bn_stats(out=st[:, 1, :], in_=y[:, NB:D])
            nc.vector.bn_aggr(out=mv_all[:, mt, :], in_=st[:])
            nc.scalar.activation(out=sd_all[:, mt:mt + 1],
                                 in_=mv_all[:, mt, 1:2],
                                 func=AF.Sqrt, bias=eps_sb[:])
            nc.vector.reciprocal(out=rstd_all[:, mt:mt + 1],
                                 in_=sd_all[:, mt:mt + 1])
            nc.vector.scalar_tensor_tensor(out=nmr_all[:, mt:mt + 1],
                                           in0=mv_all[:, mt, 0:1],
                                           scalar=-1.0,
                                           in1=rstd_all[:, mt:mt + 1],
                                           op0=ALU.mult, op1=ALU.mult)
            o = ep.tile([P, D], BF16, tag="o")
            if mt % 2 == 0:
                nc.vector.tensor_scalar(out=o[:], in0=y[:],
                                        scalar1=rstd_all[:, mt:mt + 1],
                                        scalar2=nmr_all[:, mt:mt + 1],
                                        op0=ALU.mult, op1=ALU.add)
            else:
                nc.gpsimd.tensor_scalar(out=o[:], in0=y[:],
                                        scalar1=rstd_all[:, mt:mt + 1],
                                        scalar2=nmr_all[:, mt:mt + 1],
                                        op0=ALU.mult, op1=ALU.add)
            if apply_gb:
                nc.vector.tensor_tensor(out=o[:], in0=o[:], in1=gam_sb[:],
                                        op=ALU.mult)
                nc.vector.tensor_tensor(out=o[:], in0=o[:], in1=bet_sb[:],
                                        op=ALU.add)
            dma_engs[mt % 3].dma_start(out=out.ap()[mt * P:(mt + 1) * P, :],
                                       in_=o[:])

    nc.compile()
    return nc


_NC_CACHE = {}


def _get_nc(apply_gb: bool):
    if apply_gb not in _NC_CACHE:
        _NC_CACHE[apply_gb] = _build(apply_gb)
    return _NC_CACHE[apply_gb]


def _prep_weights(Wr, att_r, Wf):
    a = np.asarray(att_r, np.float32).reshape(HD)
    at = np.tile(a, H)                            # a_d, d = 0..1023
    sg = np.where(at >= 0, 1.0, -1.0).astype(np.float32)
    sig = sg * np.maximum(np.abs(at), EPS_A) * G  # sigma_d

    WrT = np.ascontiguousarray(np.asarray(Wr, np.float32).T)   # [k, d]
    Wrp = WrT * sig[None, :]
    wr_host = np.ascontiguousarray(
        Wrp.reshape(KP, 2, P, D).transpose(2, 0, 1, 3).reshape(P, 8 * D)
    ).astype(NP8)

    # wa layout: sbuf tile [P, 8(kp*2+ks), 16] with cols 0..7 = heads
    wa_k = np.zeros((D, 16), np.float32)
    for hh in range(H):
        wa_k[:, hh] = 0.505 * G * (WrT[:, hh * HD:(hh + 1) * HD] @ a)
    wa_host = np.ascontiguousarray(
        wa_k.reshape(KP, 2, P, 16).transpose(2, 0, 1, 3).reshape(P, 8 * 16)
    ).astype(NP8)

    WfT = np.ascontiguousarray(np.asarray(Wf, np.float32).T)   # [d, n]
    Wf2 = WfT / sig[:, None]
    wf_host = np.ascontiguousarray(
        Wf2.reshape(KP, 2, P, D).transpose(2, 0, 1, 3).reshape(P, 8 * D)
    ).astype(NP8)

    sgn_host = ((0.495 / G) * sg).reshape(1, D).astype(NBF)
    return wr_host, wa_host, wf_host, sgn_host


def _make_in_maps(h, Wr, att_r, Wf, ln_gamma, ln_beta, apply_gb):
    wr_host, wa_host, wf_host, sgn_host = _prep_weights(Wr, att_r, Wf)
    hf = np.asarray(h, np.float32)                # [B, N, D]
    in_maps = []
    for c in range(NCORES):
        b, half = c // 2, c % 2
        hb = hf[b]                                # [2048, 1024]
        hT = hb.T.reshape(KP, 2, P, MT, P).transpose(3, 2, 0, 1, 4)
        hTm = np.ascontiguousarray(hT.reshape(N, D)).astype(NP8)
        m = {
            "hTm": hTm,
            "wr": wr_host,
            "wa": wa_host,
            "wf": wf_host,
            "hF": np.ascontiguousarray(
                hb[half * MTO * P:(half + 1) * MTO * P]).astype(NBF),
            "hs": np.ascontiguousarray(
                hb[half * MTO * P:(half + 1) * MTO * P].sum(axis=1)
                .reshape(MTO, P).T),
            "id8": np.eye(P, dtype=np.float32).astype(NBF),
            "sgn": sgn_host,
        }
        if apply_gb:
            m["gam"] = np.asarray(ln_gamma, np.float32).reshape(1, D)
            m["bet"] = np.asarray(ln_beta, np.float32).reshape(1, D)
        in_maps.append(m)
    return in_maps


def _run(h, Wl, Wr, att_l, att_r, Wf, ln_gamma, ln_beta, trace=False):
    g = np.asarray(ln_gamma, np.float32)
    bta = np.asarray(ln_beta, np.float32)
    apply_gb = not (np.all(g == 1.0) and np.all(bta == 0.0))
    nc = _get_nc(apply_gb)
    in_maps = _make_in_maps(h, Wr, att_r, Wf, ln_gamma, ln_beta, apply_gb)
    res = run_bass_kernel_spmd(nc, in_maps, core_ids=list(range(NCORES)),
                               trace=trace)
    outs = [np.asarray(res.results[c]["out"], np.float32) for c in range(NCORES)]
    full = np.concatenate(outs, axis=0).reshape(B, N, D)
    return full, res


def kernel(**inputs):
    out, _ = _run(**inputs)
    return out


# revision 27
# speedup vs baseline: 1.2789x; 1.0502x over previous
"""Trainium2 Bass kernel for nn_AGTLayer (GAT-style additive-attention layer).

Algebraic collapse: softmax_j(sl[i] + sr[j]) is independent of i, so the
attention reduces to one weighted mean per (batch, head):
    p[j]  = exp(sr[j]) / sum_j exp(sr[j])
    c[h]  = sum_j p[j] fr[j, h*128:(h+1)*128]
    fh    = concat_h(c[h]) @ Wf.T          (ONE vector per batch)
    out   = LayerNorm(h + fh[None, :])

Sharding: core c handles batch b = c//2 REDUNDANTLY (both cores of a pair
compute the full-batch attention; no collectives at all), then applies the
LayerNorm epilogue to its own half of the rows (half = c%2).

Speed tricks:
 - fp8(e4m3) DoubleRow matmuls everywhere on the big GEMM: K=256 per pass,
   ~2x bf16 column rate. h, Wr, Wf shipped pre-transposed/blocked in fp8.
 - att_r is folded into Wr's columns on the host (sigma_d = G*sign(a_d)*
   max(|a_d|, EPS)), so fr comes out of the matmul pre-scaled (t2 = sigma*fr);
   1/sigma is folded into Wf rows. The linear score term sum_d a_d*fr_d is an
   8-column matmul against host-computed WA; the leaky-relu |.| term is ONE
   vector stt (abs_max then * sign-const) + per-head reduce:
       sr = 0.505*sum(a*fr) + 0.495*sum(sign(a)*|t2|)/G
 - softmax denominator accumulated on-chip (sacc += w), collapsed to s with a
   single tiny matmul against ones; w = exp(sr - 1.5) (shift-invariant).
 - LayerNorm epilogue split across gpsimd/vector (adds+stats) and
   scalar/gpsimd (normalize apply); outputs stream on 3 DMA queues.
"""

import numpy as np
import ml_dtypes
from contextlib import ExitStack

import concourse.bass as bass
import concourse.mybir as mybir
import concourse.tile as tile
from concourse import bacc
from concourse.bass_utils import run_bass_kernel_spmd

AF = mybir.ActivationFunctionType
ALU = mybir.AluOpType
DR = mybir.MatmulPerfMode.DoubleRow
F32 = mybir.dt.float32
BF16 = mybir.dt.bfloat16
FP8 = mybir.dt.float8e4

B, N, D, H, HD = 4, 2048, 1024, 8, 128
NCORES = 8
P = 128
MT = N // P            # 16 row-tiles of the full batch per core
MTO = 8                # 8 output row-tiles (this core's half)
KP = D // 256          # 4 k-pairs (256 contraction per DoubleRow pass)
NB = 512               # psum bank free-dim (f32)
LN_EPS = 1e-5
G = 16.0               # column scale folded into Wr (and 1/G into Wf)
EPS_A = 0.012          # |att| clamp so Wf2 rows stay in fp8 range
W_BIAS = -1.5          # softmax shift (invariant)
NP8 = ml_dtypes.float8_e4m3
NBF = ml_dtypes.bfloat16


def _bcast_ap(ap, parts, free):
    return bass.AP(tensor=ap.tensor, offset=ap.offset, ap=[[0, parts], [1, free]])


def _build(apply_gb: bool):
    nc = bacc.Bacc(
        "TRN2",
        target_bir_lowering=False,
        debug=False,
        enable_asserts=False,
        num_devices=NCORES,
    )

    hTm = nc.dram_tensor("hTm", [N, D], FP8, kind="ExternalInput")
    wr = nc.dram_tensor("wr", [P, 8 * D], FP8, kind="ExternalInput")
    wa = nc.dram_tensor("wa", [P, 8 * 16], FP8, kind="ExternalInput")
    wf = nc.dram_tensor("wf", [P, 8 * D], FP8, kind="ExternalInput")
    hF = nc.dram_tensor("hF", [MTO * P, D], BF16, kind="ExternalInput")
    sgn = nc.dram_tensor("sgn", [P, D], BF16, kind="ExternalInput")
    hs = nc.dram_tensor("hs", [P, MTO], F32, kind="ExternalInput")
    id8 = nc.dram_tensor("id8", [P, P], BF16, kind="ExternalInput")
    out = nc.dram_tensor("out", [MTO * P, D], BF16, kind="ExternalOutput")
    if apply_gb:
        gam = nc.dram_tensor("gam", [1, D], F32, kind="ExternalInput")
        bet = nc.dram_tensor("bet", [1, D], F32, kind="ExternalInput")

    with tile.TileContext(nc) as tc, ExitStack() as ctx:
        const = ctx.enter_context(tc.tile_pool(name="const", bufs=1))
        work = ctx.enter_context(tc.tile_pool(name="work", bufs=3))
        ep = ctx.enter_context(tc.tile_pool(name="ep", bufs=4))
        eps_p = ctx.enter_context(tc.tile_pool(name="eps", bufs=4))
        frp = ctx.enter_context(tc.tile_pool(name="frp", bufs=2, space="PSUM"))
        stp = ctx.enter_context(tc.tile_pool(name="stp", bufs=2, space="PSUM"))
        up = ctx.enter_context(tc.tile_pool(name="up", bufs=1, space="PSUM"))
        dram = ctx.enter_context(tc.tile_pool(name="dram", bufs=1, space="DRAM"))

        # ---- tiny constants ----
        warm_w = const.tile([P, NB], BF16, tag="warm_w")
        nc.vector.memset(warm_w[:], 0.0)
        ones_m = const.tile([P, 1], BF16, tag="ones_m")
        nc.vector.memset(ones_m[:], 1.0)
        ones1b = const.tile([1, P], BF16, tag="ones1b")
        nc.vector.memset(ones1b[:], 1.0)
        eps_sb = const.tile([P, 1], F32, tag="eps")
        nc.vector.memset(eps_sb[:], LN_EPS)
        sacc = const.tile([P, H], F32, tag="sacc")
        nc.gpsimd.memset(sacc[:], 0.0)
        wb_sb = const.tile([P, 1], F32, tag="wb")
        nc.vector.memset(wb_sb[:], W_BIAS)
        zero_sb = const.tile([P, 1], F32, tag="zero")
        nc.vector.memset(zero_sb[:], 0.0)
        un_sb = const.tile([P, D], BF16, tag="un_sb")
        nc.gpsimd.memset(un_sb[:], 0.0)

        # ---- PE warmup burst: unthrottle HAM while first DMAs land ----
        warm_ps = frp.tile([P, D], F32, tag="fr")
        NWARM = 9
        for i in range(NWARM):
            nc.tensor.matmul(warm_ps[:, 0:NB], lhsT=warm_w[:, 0:P], rhs=warm_w[:],
                             start=(i == 0), stop=(i == NWARM - 1))

        # ---- input loads, spread across DGE queues ----
        # critical path: wr_kp0 halves + htm0 + wa + sgn feed tile 0.
        wr_t, wf_t, htm, h_t = [], [], [], []
        for kp in range(KP):
            wr_t.append(const.tile([P, 2, D], FP8, tag=f"wr{kp}", name=f"wr{kp}"))
            wf_t.append(const.tile([P, 2, D], FP8, tag=f"wf{kp}", name=f"wf{kp}"))
        for mt in range(MT):
            htm.append(const.tile([P, H * P], FP8, tag=f"htm{mt}", name=f"htm{mt}"))
        for mt in range(MTO):
            h_t.append(const.tile([P, D], BF16, tag=f"h{mt}", name=f"h{mt}"))
        wa_sb = const.tile([P, 8 * 16], FP8, tag="wa")
        hs_sb = const.tile([P, MTO], F32, tag="hs")
        id_sb = const.tile([P, P], BF16, tag="id8")
        sgn_b = const.tile([P, D], BF16, tag="sgn")

        wr_src = wr.ap().rearrange("p (kp ks n) -> p kp ks n", kp=KP, ks=2)
        wf_src = wf.ap().rearrange("p (kp ks n) -> p kp ks n", kp=KP, ks=2)
        # tile 0 needs ALL wr chunks + htm0 + wa + sgn: front-load those on
        # all 3 queues, then stream htm in consumption order, then wf/hF.
        engs = [nc.sync, nc.scalar, nc.gpsimd]
        nc.sync.dma_start(out=wr_t[0][:], in_=wr_src[:, 0])
        nc.scalar.dma_start(out=htm[0][:], in_=hTm.ap()[0:P, :])
        nc.gpsimd.dma_start(out=wa_sb[:], in_=wa.ap())
        nc.gpsimd.dma_start(out=wr_t[3][:], in_=wr_src[:, 3])
        nc.sync.dma_start(out=wr_t[1][:], in_=wr_src[:, 1])
        nc.scalar.dma_start(out=wr_t[2][:], in_=wr_src[:, 2])
        nc.gpsimd.dma_start(out=htm[1][:], in_=hTm.ap()[P:2 * P, :])
        nc.scalar.dma_start(out=sgn_b[:], in_=sgn.ap())
        for mt in range(2, MT):
            engs[mt % 3].dma_start(out=htm[mt][:], in_=hTm.ap()[mt * P:(mt + 1) * P, :])
        nc.sync.dma_start(out=hs_sb[:], in_=hs.ap())
        nc.scalar.dma_start(out=id_sb[:], in_=id8.ap())
        for kp in range(KP):
            engs[kp % 3].dma_start(out=wf_t[kp][:], in_=wf_src[:, kp])
        for mt in range(MTO):
            engs[(mt + 1) % 3].dma_start(out=h_t[mt][:], in_=hF.ap()[mt * P:(mt + 1) * P, :])
        if apply_gb:
            gam_sb = const.tile([P, D], F32, tag="gam")
            nc.sync.dma_start(out=gam_sb[:], in_=_bcast_ap(gam.ap(), P, D))
            bet_sb = const.tile([P, D], F32, tag="bet")
            nc.sync.dma_start(out=bet_sb[:], in_=_bcast_ap(bet.ap(), P, D))

        # ---- main loop: fr + scores, u accumulated one tile behind ----
        frb2 = [const.tile([P, 2, D], FP8, tag=f"frb{p}", name=f"frb{p}") for p in range(MT // 2)]
        w2 = [const.tile([P, 2, 16], FP8, tag=f"w{p}", name=f"w{p}") for p in range(MT // 2)]
        u_ps = up.tile([H, D], F32, tag="u")

        def u_mms(pr):
            lw = w2[pr][:, :, 0:H]
            for nh in range(2):
                nc.tensor.matmul(
                    u_ps[0:H, nh * NB:(nh + 1) * NB],
                    lhsT=lw,
                    rhs=frb2[pr][:, :, nh * NB:(nh + 1) * NB],
                    start=(pr == 0), stop=(pr == MT // 2 - 1),
                    perf_mode=DR,
                )

        for step in range(MT + 1):
            if step < MT:
                mt = step
                pr, ko = mt // 2, mt % 2
                fr = frp.tile([P, D], F32, tag="fr")
                sT = stp.tile([P, H], F32, tag="sT")
                htm_r = htm[mt][:].rearrange("p (ks j) -> p ks j", ks=H)
                wa_r = wa_sb[:].rearrange("p (s c) -> p s c", c=16)
                for kp in range(KP):
                    lhs = htm_r[:, 2 * kp:2 * kp + 2, :]
                    for nh in range(2):
                        nc.tensor.matmul(
                            fr[:, nh * NB:(nh + 1) * NB],
                            lhsT=lhs,
                            rhs=wr_t[kp][:, :, nh * NB:(nh + 1) * NB],
                            start=(kp == 0), stop=(kp == KP - 1),
                            perf_mode=DR,
                        )
                    nc.tensor.matmul(
                        sT[:, 0:H],
                        lhsT=lhs,
                        rhs=wa_r[:, 2 * kp:2 * kp + 2, 0:H],
                        start=(kp == 0), stop=(kp == KP - 1),
                        perf_mode=DR,
                    )
                # evac fr -> fp8 (scalar), |t2|*sgn (gpsimd/vector), reduce,
                # combine with linear term, exp -> w (scalar), sacc += w.
                nc.scalar.activation(out=frb2[pr][:, ko, :], in_=fr[:], func=AF.Copy)
                m = work.tile([P, D], BF16, tag="m")
                last = mt >= MT - 2
                if mt % 2 == 0 or last:
                    nc.scalar.activation(out=m[:], in_=fr[:], func=AF.Abs,
                                         bias=zero_sb[:, 0:1])
                else:
                    fb = frb2[pr][:, ko, :]
                    nc.vector.scalar_tensor_tensor(out=m[:], in0=fb,
                                                   scalar=-1.0, in1=fb,
                                                   op0=ALU.mult, op1=ALU.max)
                ms = work.tile([P, D], BF16, tag="ms")
                mseng = nc.vector if last else nc.gpsimd
                mseng.tensor_tensor(out=ms[:], in0=m[:], in1=sgn_b[:],
                                    op=ALU.mult)
                S = work.tile([P, H], F32, tag="S")
                nc.vector.tensor_reduce(
                    out=S[:],
                    in_=ms[:].rearrange("p (h hd) -> p h hd", h=H),
                    axis=mybir.AxisListType.X,
                    op=ALU.add,
                )
                sr = work.tile([P, H], F32, tag="sr")
                nc.vector.scalar_tensor_tensor(out=sr[:], in0=sT[:],
                                               scalar=1.0 / G, in1=S[:],
                                               op0=ALU.mult, op1=ALU.add)
                nc.scalar.activation(out=w2[pr][:, ko, 0:H], in_=sr[:],
                                     func=AF.Exp, bias=wb_sb[:, 0:1])
                nc.vector.tensor_tensor(out=sacc[:], in0=sacc[:],
                                        in1=w2[pr][:, ko, 0:H], op=ALU.add)
            if step >= 2 and step % 2 == 0:
                u_mms((step - 2) // 2)
        u_mms(MT // 2 - 1)

        # ---- softmax denominator + normalized context ----
        sacc_bf = const.tile([P, H], BF16, tag="sacc_bf")
        nc.vector.tensor_copy(out=sacc_bf[:], in_=sacc[:])
        s_ps = stp.tile([H, 1], F32, tag="sT")
        nc.tensor.matmul(s_ps[:], lhsT=sacc_bf[:], rhs=ones_m[:],
                         start=True, stop=True)
        rs = const.tile([H, 1], F32, tag="rs")
        nc.vector.reciprocal(out=rs[:], in_=s_ps[:])
        nc.vector.tensor_scalar(out=un_sb[0:H, :], in0=u_ps[:],
                                scalar1=rs[:, 0:1], scalar2=None, op0=ALU.mult)
        # transpose each 128-block of un via PE; head h's own block is column
        # h of transpose #h. Assemble fp8 lhsT tiles for the fh matmuls.
        cn_t = [const.tile([P, 2 * 16], FP8, tag=f"cn{kp}", name=f"cn{kp}")
                for kp in range(KP)]
        for hh in range(H):
            tp = stp.tile([P, P], BF16, tag="sT")
            nc.tensor.transpose(out=tp[:], in_=un_sb[:, hh * HD:(hh + 1) * HD],
                                identity=id_sb[:])
            dst = cn_t[hh // 2][:].rearrange("p (k s) -> p k s", s=16)
            nc.vector.tensor_copy(out=dst[:, hh % 2, 0:1], in_=tp[:, hh:hh + 1])

        # ---- fh = c @ Wf2, broadcast to 128 rows ----
        fh_ps = frp.tile([1, D], F32, tag="fr")
        for kp in range(KP):
            lhs = cn_t[kp][:].rearrange("p (k s) -> p k s", s=16)[:, :, 0:1]
            for nh in range(2):
                nc.tensor.matmul(
                    fh_ps[0:1, nh * NB:(nh + 1) * NB],
                    lhsT=lhs,
                    rhs=wf_t[kp][:, :, nh * NB:(nh + 1) * NB],
                    start=(kp == 0), stop=(kp == KP - 1),
                    perf_mode=DR,
                )
        fh_sb = const.tile([1, D], BF16, tag="fh_sb")
        nc.vector.tensor_copy(out=fh_sb[:], in_=fh_ps[:])
        fhb_ps = frp.tile([P, D], F32, tag="fr")
        for nh in range(2):
            nc.tensor.matmul(fhb_ps[:, nh * NB:(nh + 1) * NB], lhsT=ones1b[:],
                             rhs=fh_sb[0:1, nh * NB:(nh + 1) * NB],
                             start=True, stop=True)
        fhb = const.tile([P, D], BF16, tag="fhb")
        nc.scalar.activation(out=fhb[:, 0:NB], in_=fhb_ps[:, 0:NB], func=AF.Copy)
        nc.vector.tensor_copy(out=fhb[:, NB:D], in_=fhb_ps[:, NB:D])

        # ---- epilogue: Sum(y) = host Sum(h) + Sum(fh); Sum(y^2) via scalar
        # Square+accum; per-tile rstd; applies split vector/gpsimd ----
        fsum = eps_p.tile([P, 1], F32, tag="fsum")
        nc.vector.tensor_reduce(out=fsum[:], in_=fhb[:],
                                axis=mybir.AxisListType.X, op=ALU.add)
        ysum = eps_p.tile([P, MTO], F32, tag="ysum")
        nc.vector.tensor_scalar(out=ysum[:], in0=hs_sb[:], scalar1=fsum[:, 0:1],
                                scalar2=None, op0=ALU.add)
        mu_all = eps_p.tile([P, MTO], F32, tag="mu_all")
        nc.vector.tensor_scalar(out=mu_all[:], in0=ysum[:], scalar1=1.0 / D,
                                scalar2=None, op0=ALU.mult)
        y2sum = eps_p.tile([P, MTO], F32, tag="y2sum")
        var_all = eps_p.tile([P, MTO], F32, tag="var_all")
        sd_all = eps_p.tile([P, MTO], F32, tag="sd_all")
        rstd_all = eps_p.tile([P, MTO], F32, tag="rstd_all")
        nmr_all = eps_p.tile([P, MTO], F32, tag="nmr_all")
        dma_engs = [nc.sync, nc.gpsimd, nc.scalar]
        for mt in range(MTO):
            y = ep.tile([P, D], BF16, tag=f"y{mt % 4}")
            aeng = nc.gpsimd if mt % 4 == 1 else nc.vector
            aeng.tensor_tensor(out=y[:], in0=h_t[mt][:], in1=fhb[:],
                               op=ALU.add)
            sq = work.tile([P, D], BF16, tag="sq")
            nc.scalar.activation(out=sq[:], in_=y[:], func=AF.Square,
                                 bias=zero_sb[:, 0:1],
                                 accum_out=y2sum[:, mt:mt + 1])
            nc.vector.scalar_tensor_tensor(out=var_all[:, mt:mt + 1],
                                           in0=mu_all[:, mt:mt + 1],
                                           scalar=-1.0,
                                           in1=mu_all[:, mt:mt + 1],
                                           op0=ALU.mult, op1=ALU.mult)
            nc.vector.scalar_tensor_tensor(out=var_all[:, mt:mt + 1],
                                           in0=y2sum[:, mt:mt + 1],
                                           scalar=1.0 / D,
                                           in1=var_all[:, mt:mt + 1],
                                           op0=ALU.mult, op1=ALU.add)
            nc.scalar.activation(out=sd_all[:, mt:mt + 1],
                                 in_=var_all[:, mt:mt + 1],
                                 func=AF.Sqrt, bias=eps_sb[:])
            nc.vector.reciprocal(out=rstd_all[:, mt:mt + 1],
                                 in_=sd_all[:, mt:mt + 1])
            nc.vector.scalar_tensor_tensor(out=nmr_all[:, mt:mt + 1],
                                           in0=mu_all[:, mt:mt + 1],
                                           scalar=-1.0,
                                           in1=rstd_all[:, mt:mt + 1],
                                           op0=ALU.mult, op1=ALU.mult)
            o = ep.tile([P, D], BF16, tag="o")
            oeng = nc.vector if mt % 4 == 3 else nc.gpsimd
            oeng.tensor_scalar(out=o[:], in0=y[:],
                               scalar1=rstd_all[:, mt:mt + 1],
                               scalar2=nmr_all[:, mt:mt + 1],
                               op0=ALU.mult, op1=ALU.add)
            if apply_gb:
                nc.vector.tensor_tensor(out=o[:], in0=o[:], in1=gam_sb[:],
                                        op=ALU.mult)
                nc.vector.tensor_tensor(out=o[:], in0=o[:], in1=bet_sb[:],
                                        op=ALU.add)
            dma_engs[mt % 3].dma_start(out=out.ap()[mt * P:(mt + 1) * P, :],
                                       in_=o[:])

    nc.compile()
    return nc


_NC_CACHE = {}


def _get_nc(apply_gb: bool):
    if apply_gb not in _NC_CACHE:
        _NC_CACHE[apply_gb] = _build(apply_gb)
    return _NC_CACHE[apply_gb]


def _prep_weights(Wr, att_r, Wf):
    a = np.asarray(att_r, np.float32).reshape(HD)
    at = np.tile(a, H)                            # a_d, d = 0..1023
    sg = np.where(at >= 0, 1.0, -1.0).astype(np.float32)
    sig = sg * np.maximum(np.abs(at), EPS_A) * G  # sigma_d

    WrT = np.ascontiguousarray(np.asarray(Wr, np.float32).T)   # [k, d]
    Wrp = WrT * sig[None, :]
    wr_host = np.ascontiguousarray(
        Wrp.reshape(KP, 2, P, D).transpose(2, 0, 1, 3).reshape(P, 8 * D)
    ).astype(NP8)

    # wa layout: sbuf tile [P, 8(kp*2+ks), 16] with cols 0..7 = heads
    wa_k = np.zeros((D, 16), np.float32)
    for hh in range(H):
        wa_k[:, hh] = 0.505 * G * (WrT[:, hh * HD:(hh + 1) * HD] @ a)
    wa_host = np.ascontiguousarray(
        wa_k.reshape(KP, 2, P, 16).transpose(2, 0, 1, 3).reshape(P, 8 * 16)
    ).astype(NP8)

    WfT = np.ascontiguousarray(np.asarray(Wf, np.float32).T)   # [d, n]
    Wf2 = WfT / sig[:, None]
    wf_host = np.ascontiguousarray(
        Wf2.reshape(KP, 2, P, D).transpose(2, 0, 1, 3).reshape(P, 8 * D)
    ).astype(NP8)

    sgn_host = np.ascontiguousarray(np.broadcast_to(
        ((0.495 / G) * sg).reshape(1, D), (P, D))).astype(NBF)
    return wr_host, wa_host, wf_host, sgn_host


def _make_in_maps(h, Wr, att_r, Wf, ln_gamma, ln_beta, apply_gb):
    wr_host, wa_host, wf_host, sgn_host = _prep_weights(Wr, att_r, Wf)
    hf = np.asarray(h, np.float32)                # [B, N, D]
    in_maps = []
    for c in range(NCORES):
        b, half = c // 2, c % 2
        hb = hf[b]                                # [2048, 1024]
        hT = hb.T.reshape(KP, 2, P, MT, P).transpose(3, 2, 0, 1, 4)
        hTm = np.ascontiguousarray(hT.reshape(N, D)).astype(NP8)
        m = {
            "hTm": hTm,
            "wr": wr_host,
            "wa": wa_host,
            "wf": wf_host,
            "hF": np.ascontiguousarray(
                hb[half * MTO * P:(half + 1) * MTO * P]).astype(NBF),
            "hs": np.ascontiguousarray(
                hb[half * MTO * P:(half + 1) * MTO * P].sum(axis=1)
                .reshape(MTO, P).T),
            "id8": np.eye(P, dtype=np.float32).astype(NBF),
            "sgn": sgn_host,
        }
        if apply_gb:
            m["gam"] = np.asarray(ln_gamma, np.float32).reshape(1, D)
            m["bet"] = np.asarray(ln_beta, np.float32).reshape(1, D)
        in_maps.append(m)
    return in_maps


def _run(h, Wl, Wr, att_l, att_r, Wf, ln_gamma, ln_beta, trace=False):
    g = np.asarray(ln_gamma, np.float32)
    bta = np.asarray(ln_beta, np.float32)
    apply_gb = not (np.all(g == 1.0) and np.all(bta == 0.0))
    nc = _get_nc(apply_gb)
    in_maps = _make_in_maps(h, Wr, att_r, Wf, ln_gamma, ln_beta, apply_gb)
    res = run_bass_kernel_spmd(nc, in_maps, core_ids=list(range(NCORES)),
                               trace=trace)
    outs = [np.asarray(res.results[c]["out"], np.float32) for c in range(NCORES)]
    full = np.concatenate(outs, axis=0).reshape(B, N, D)
    return full, res


def kernel(**inputs):
    out, _ = _run(**inputs)
    return out
